# revision 1
# baseline (speedup 1.0000x reference)
"""TRN2 Bass kernel for nn_ClassSemantic (scatter_memory).

Strategy
--------
Data-parallel over batch: core k owns samples 4k..4k+3 and runs
projection (1x1 conv) + memory-gather attention + concat, all in fp32r
(TF32-like, ~13 mantissa bits, 4x faster PE than fp32).

The sequential EMA queue update depends on the per-sample masked
feature means only, which are algebraically separable:
    feat_b = mean_hw((Wp@f + bp) * pred) = Wp @ mean_hw(f * pred) + bp * mean(pred)
The inner reduction (134 MFLOP, 0.4% of total work) is computed on the
host, then the exactly-sequential 32-step EMA scan (tiny: [4,20,256]
state) runs on the host in float64 and the final queue rows are shipped
to every core as constants.  The device never needs a collective.

Softmax over the 20 memory slots: logits are empirically in [-3, 3]
(queue rows are ~unit-norm, x ~ N(0,1)), so exp() without max
subtraction is safe.  Column sums / broadcasts across the 20-partition
axis are done with tiny ones-matmuls on the PE.
"""
import os
import numpy as np
from contextlib import ExitStack

B, IN_C, H, W_SP = 32, 512, 64, 64
CODE, CLASSES, MEM = 256, 4, 20
HW = H * W_SP              # 4096
NCORES = 8
BPC = B // NCORES          # 4 samples per core
DECAY, EPS = 0.9, 1e-12
NCH = 8                    # n-chunks per sample
NT = HW // NCH             # 512 spatial positions per chunk

_PROGRAM_CACHE = {}
LAST_RESULTS = None        # stash for test harness introspection


def _host_queue_update(feats, preds, labels, flag, queue, Wp, bp):
    """Final queue after the reference's sequential EMA scan (float64)."""
    if int(flag) != 1:
        return queue.astype(np.float32)
    f3 = feats.reshape(B, IN_C, HW)
    p2 = preds.reshape(B, HW)
    # g_b = mean_n feats_b[:, n] * pred_b[n]  (batched sgemv)
    g = np.matmul(f3, p2[:, :, None])[:, :, 0] / np.float32(HW)
    feat = g @ Wp.T.astype(np.float32) + bp[None, :] * p2.mean(axis=1)[:, None]
    q = queue.astype(np.float64)
    for i in range(B):
        l = int(labels[i])
        f = feat[i].astype(np.float64)
        slot = q[l]
        logit = slot @ f
        upd = logit[:, None] * f[None, :]
        nrm = np.sqrt((upd * upd).sum(axis=1, keepdims=True))
        upd = upd / np.maximum(nrm, EPS)
        q[l] = DECAY * slot + (1.0 - DECAY) * upd
    return q.astype(np.float32)


def _build_program():
    from concourse import bacc, mybir
    import concourse.tile as tile

    f32, f32r = mybir.dt.float32, mybir.dt.float32r
    nc = bacc.Bacc("TRN2", target_bir_lowering=False, debug=False)

    feats_in = nc.dram_tensor("feats", [BPC, IN_C, HW], f32r, kind="ExternalInput").ap()
    wpt_in = nc.dram_tensor("wpt", [IN_C, CODE], f32r, kind="ExternalInput").ap()
    bp_in = nc.dram_tensor("bpc", [128, 2], f32, kind="ExternalInput").ap()
    qat_in = nc.dram_tensor("qat", [BPC, CODE, MEM], f32r, kind="ExternalInput").ap()
    qa_in = nc.dram_tensor("qa", [BPC, MEM, CODE], f32r, kind="ExternalInput").ap()
    ones20_in = nc.dram_tensor("ones20", [MEM, MEM], f32r, kind="ExternalInput").ap()
    out_ext = nc.dram_tensor("out", [BPC, 2 * CODE, HW], f32, kind="ExternalOutput").ap()

    with tile.TileContext(nc) as tc, ExitStack() as ctx:
        consts = ctx.enter_context(tc.tile_pool(name="consts", bufs=1))
        fpool = ctx.enter_context(tc.tile_pool(name="fpool", bufs=4))
        xpool = ctx.enter_context(tc.tile_pool(name="xpool", bufs=2))
        upool = ctx.enter_context(tc.tile_pool(name="upool", bufs=2))
        spool = ctx.enter_context(tc.tile_pool(name="spool", bufs=2))
        spool4 = ctx.enter_context(tc.tile_pool(name="spool4", bufs=5))
        ppp = ctx.enter_context(tc.tile_pool(name="ppp", bufs=2, space="PSUM"))
        pps = ctx.enter_context(tc.tile_pool(name="pps", bufs=2, space="PSUM"))
        ppc = ctx.enter_context(tc.tile_pool(name="ppc", bufs=2, space="PSUM"))
        ppu = ctx.enter_context(tc.tile_pool(name="ppu", bufs=2, space="PSUM"))

        # constants load on the scalar HWDGE ring so the sync ring starts
        # streaming feats immediately
        wpt_sb = consts.tile([128, 4, CODE], f32r, name="wpt_sb")       # [p, kchunk, o]
        nc.scalar.dma_start(wpt_sb[:], wpt_in.rearrange("(kk p) m -> p kk m", p=128))
        bp_sb = consts.tile([128, 2], f32, name="bp_sb")                # [p, half]
        nc.scalar.dma_start(bp_sb[:], bp_in[:])
        qat_sb = consts.tile([128, BPC, 2, MEM], f32r, name="qat_sb")   # [p, b, kchunk, m]
        qa_sb = consts.tile([MEM, BPC, CODE], f32r, name="qa_sb")       # [m, b, c]
        ones20_sb = consts.tile([MEM, MEM], f32r, name="ones20_sb")

        def load_attn_consts():
            nc.scalar.dma_start(qat_sb[:], qat_in.rearrange("b (kk p) m -> p b kk m", p=128))
            nc.scalar.dma_start(qa_sb[:], qa_in.rearrange("b m c -> m b c"))
            nc.scalar.dma_start(ones20_sb[:], ones20_in[:])

        x_tiles = {}
        u_tiles = {}
        pexp_t = {}
        cs_t = {}
        pn_t = {}
        T = BPC * NCH

        def bj(c):
            return c // NCH, c % NCH

        def proj_chunk(c):
            b, j = bj(c)
            if j == 0:
                x_tiles[b] = xpool.tile([128, 2, HW], f32r, tag="x_sb", name=f"x_sb{b}")
                u_tiles[b] = upool.tile([128, 2, HW], f32, tag="u_sb", name=f"u_sb{b}")
            feats_b = feats_in[b].rearrange("(kk p) n -> p kk n", p=128)
            x_sb = x_tiles[b]
            ft = fpool.tile([128, 4, NT], f32r, tag="ft", name=f"ft{c}")
            nc.sync.dma_start(ft[:], feats_b[:, :, j * NT:(j + 1) * NT])
            for h in range(2):
                ps = ppp.tile([128, NT], f32, tag="proj_ps", name=f"pps{c}_{h}")
                for kk in range(4):
                    nc.tensor.matmul(
                        ps[:], wpt_sb[:, kk, h * 128:(h + 1) * 128], ft[:, kk, :],
                        start=(kk == 0), stop=(kk == 3))
                # psum -> sbuf with per-channel bias; fp32r out rounds for PE
                if h == 0:
                    nc.scalar.activation(
                        x_sb[:, h, j * NT:(j + 1) * NT], ps[:],
                        mybir.ActivationFunctionType.Identity,
                        bias=bp_sb[:, h:h + 1])
                else:
                    nc.vector.tensor_scalar_add(
                        x_sb[:, h, j * NT:(j + 1) * NT], ps[:], bp_sb[:, h:h + 1])

        def logit_stage(c):
            b, j = bj(c)
            x_sb = x_tiles[b]
            js = slice(j * NT, (j + 1) * NT)
            lg = pps.tile([MEM, NT], f32, tag="logit_ps", name=f"lg{c}")
            for kk in range(2):
                nc.tensor.matmul(lg[:], qat_sb[:, b, kk, :], x_sb[:, kk, js],
                                 start=(kk == 0), stop=(kk == 1))
            pexp = spool4.tile([MEM, NT], f32r, tag="pexp", name=f"pexp{c}")
            nc.scalar.activation(pexp[:], lg[:], mybir.ActivationFunctionType.Exp)
            pexp_t[c] = pexp

        def sum_stage(c):
            # lhsT = all-ones [20,20]: every output partition gets the
            # column sum, so no cross-partition broadcast is needed later.
            cs = ppc.tile([MEM, NT], f32, tag="colsum_ps", name=f"cs{c}")
            nc.tensor.matmul(cs[:], ones20_sb[:], pexp_t[c][:], start=True, stop=True)
            cs_t[c] = cs

        def recip_stage(c):
            # 1/colsum at ~18 correct bits (more than fp32r's mantissa)
            rc = spool.tile([MEM, NT], f32, tag="recip", name=f"rc{c}")
            nc.vector.reciprocal_approx_fast(out=rc[:], in_=cs_t.pop(c)[:])
            pn_t[c] = (rc,)

        def u_stage(c):
            b, j = bj(c)
            u_sb = u_tiles[b]
            js = slice(j * NT, (j + 1) * NT)
            (rc,) = pn_t.pop(c)
            pn = spool.tile([MEM, NT], f32r, tag="pn", name=f"pn{c}")
            nc.vector.tensor_mul(pn[:], pexp_t.pop(c)[:], rc[:])
            for h in range(2):
                us = ppu.tile([128, NT], f32, tag="u_ps", name=f"us{c}_{h}")
                nc.tensor.matmul(us[:], qa_sb[:, b, h * 128:(h + 1) * 128], pn[:],
                                 start=True, stop=True)
                if h == 0:
                    nc.scalar.copy(u_sb[:, h, js], us[:])
                else:
                    nc.vector.tensor_copy(u_sb[:, h, js], us[:])

        def x_flush(c):
            # stream the x half of the output out chunk-by-chunk to keep the
            # write bandwidth demand even across the kernel
            b, j = bj(c)
            x_sb = x_tiles[b]
            js = slice(j * NT, (j + 1) * NT)
            for h in range(2):
                nc.gpsimd.dma_start(
                    out_ext[b, 256 + h * 128:256 + (h + 1) * 128, j * NT:(j + 1) * NT],
                    x_sb[:, h, js].bitcast(f32))

        def u_flush(c):
            b, j = bj(c)
            u_sb = u_tiles[b]
            js = slice(j * NT, (j + 1) * NT)
            for h in range(2):
                nc.gpsimd.dma_start(out_ext[b, h * 128:(h + 1) * 128, j * NT:(j + 1) * NT],
                                    u_sb[:, h, js])

        # Chunk-level software pipeline: stage s of chunk c is emitted at
        # iteration c+s, so every cross-engine hop has a full iteration of
        # slack and the PE stream never waits on the softmax chain.
        for t in range(T + 6):
            if t < T:
                proj_chunk(t)
            if t == 0:
                load_attn_consts()
            if 0 <= t - 1 < T:
                logit_stage(t - 1)
            if 0 <= t - 2 < T:
                sum_stage(t - 2)
                x_flush(t - 2)
            if 0 <= t - 3 < T:
                recip_stage(t - 3)
            if 0 <= t - 4 < T:
                u_stage(t - 4)
            if 0 <= t - 5 < T:
                u_flush(t - 5)

    nc.compile()
    return nc


def kernel(feats, preds, labels, flag, queue, Wp, bp):
    from concourse.bass_utils import run_bass_kernel_spmd
    global LAST_RESULTS

    feats = np.ascontiguousarray(np.asarray(feats, dtype=np.float32))
    preds = np.ascontiguousarray(np.asarray(preds, dtype=np.float32))
    labels = np.asarray(labels).astype(np.int64)
    queue = np.ascontiguousarray(np.asarray(queue, dtype=np.float32))
    Wp = np.ascontiguousarray(np.asarray(Wp, dtype=np.float32))
    bp = np.ascontiguousarray(np.asarray(bp, dtype=np.float32))
    try:
        flag_v = int(np.asarray(flag))
    except TypeError:
        flag_v = int(flag)

    qfin = _host_queue_update(feats, preds, labels, flag_v, queue, Wp, bp)
    qA = np.ascontiguousarray(qfin[labels])                      # [B, 20, 256]
    qAT = np.ascontiguousarray(qA.transpose(0, 2, 1))            # [B, 256, 20]
    wpt = np.ascontiguousarray(Wp.T)                             # [512, 256]
    bpc = np.ascontiguousarray(bp.reshape(2, 128).T)
    ones20 = np.ones((MEM, MEM), dtype=np.float32)

    if "prog" not in _PROGRAM_CACHE:
        _PROGRAM_CACHE["prog"] = _build_program()
    nc = _PROGRAM_CACHE["prog"]

    f4 = feats.reshape(B, IN_C, HW)
    in_maps = []
    for k in range(NCORES):
        s = slice(k * BPC, (k + 1) * BPC)
        in_maps.append({
            "feats": np.ascontiguousarray(f4[s]),
            "wpt": wpt,
            "bpc": bpc,
            "qat": np.ascontiguousarray(qAT[s]),
            "qa": np.ascontiguousarray(qA[s]),
            "ones20": ones20,
        })

    trace = bool(int(os.environ.get("KERNEL_TRACE", "0")))
    tc_env = os.environ.get("KERNEL_TRACE_CORES", "")
    trace_cores = [int(x) for x in tc_env.split(",") if x] or None
    res = run_bass_kernel_spmd(nc, in_maps, core_ids=list(range(NCORES)),
                               trace=trace, trace_cores=trace_cores)
    LAST_RESULTS = res
    out = np.concatenate([res.results[k]["out"] for k in range(NCORES)], axis=0)
    return out.reshape(B, 2 * CODE, H, W_SP)


if __name__ == "__main__":
    d = np.load("/tmp/inputs.npz")
    out = kernel(d["feats"], d["preds"], d["labels"], d["flag"], d["queue"], d["Wp"], d["bp"])
    exp = np.load("/tmp/expected.npy")
    err = np.abs(out - exp)
    print("absmax err:", err.max(), "scale-rel:", err.max() / np.abs(exp).max())



# revision 15
# speedup vs baseline: 1.2487x; 1.2487x over previous
"""TRN2 Bass kernel for nn_ClassSemantic (scatter_memory).

Strategy (v2)
-------------
Data-parallel over batch: core k owns samples 4k..4k+3 and runs
projection (1x1 conv) + memory-gather attention + concat on device.

The kernel is memory-regime: the fp32 baseline was at ~95% of the DMA
roofline (67 MB/core).  v2 cuts HBM bytes ~2.3x:
  - feats are sent as fp16 (host converts; proj error ~5e-4 rel)
  - the x output half is written as fp16
  - the attention output half u is written as fp8-e4m3 (|u| <= 0.08
    while the output scale is 5.7, so fp8 error is ~1e-4 of scale)
Host packs feats into the exact per-chunk SBUF layout so each chunk is
ONE contiguous 512KB DMA descriptor (descriptor issue costs ~0.7us of
engine time each; baseline used 5/chunk, v2 uses 3/chunk).

With bytes halved the PE becomes the critical engine (~84us); the
attention path is arranged to minimize PE column passes:
  - logits per chunk land in [20, NT] PSUM at base partition 0; the
    exp ACTIVATE partition-shifts its write into a persistent
    [128, NT] tile at offset 32*(c%3), batching 3 chunks per group
    (engines are column-bound, so ops on [20, NT] cost the same as
    [128, NT]; matmuls may NOT write PSUM at partition offsets, but
    engine writes and matmul READS at offsets 0/32/64 are legal)
  - the softmax column-sum for a whole group is ONE [128,128]
    block-diagonal ones matmul (rows outside the 20 valid slots of
    each 32-row strip are zeroed by a one-time zeros DMA)
  - recip + attn-normalize run once per group on [128, NT]
  - the u matmuls contract the 20-row strip at offset 32*(c%3)

The sequential EMA queue update depends only on per-sample masked
feature means, computed on host (0.4% of FLOPs), and the final queue
rows are shipped to every core as constants; no collective needed.
"""
import os
import numpy as np
import ml_dtypes
from contextlib import ExitStack

B, IN_C, H, W_SP = 32, 512, 64, 64
CODE, CLASSES, MEM = 256, 4, 20
HW = H * W_SP              # 4096
NCORES = 8
BPC = B // NCORES          # 4 samples per core
DECAY, EPS = 0.9, 1e-12
NCH = 8                    # chunks per sample
NT = HW // NCH             # 512 spatial positions per chunk
T = BPC * NCH              # 32 chunks per core
GSZ = 3                    # chunks per softmax group (offsets 0/32/64)
NG = (T + GSZ - 1) // GSZ
F8 = ml_dtypes.float8_e4m3

_PROGRAM_CACHE = {}
LAST_RESULTS = None        # stash for test harness introspection


def _host_queue_update(feats, preds, labels, flag, queue, Wp, bp):
    """Final queue after the reference's sequential EMA scan (float64)."""
    if int(flag) != 1:
        return queue.astype(np.float32)
    f3 = feats.reshape(B, IN_C, HW)
    p2 = preds.reshape(B, HW)
    g = np.matmul(f3, p2[:, :, None])[:, :, 0] / np.float32(HW)
    feat = g @ Wp.T.astype(np.float32) + bp[None, :] * p2.mean(axis=1)[:, None]
    q = queue.astype(np.float64)
    for i in range(B):
        l = int(labels[i])
        f = feat[i].astype(np.float64)
        slot = q[l]
        logit = slot @ f
        upd = logit[:, None] * f[None, :]
        nrm = np.sqrt((upd * upd).sum(axis=1, keepdims=True))
        upd = upd / np.maximum(nrm, EPS)
        q[l] = DECAY * slot + (1.0 - DECAY) * upd
    return q.astype(np.float32)


def _build_program():
    from concourse import bacc, mybir
    import concourse.tile as tile

    f32, f32r = mybir.dt.float32, mybir.dt.float32r
    f16, f8 = mybir.dt.float16, mybir.dt.float8e4
    nc = bacc.Bacc("TRN2", target_bir_lowering=False, debug=False)

    feats_in = nc.dram_tensor("feats", [T, 128, 4, NT], f16, kind="ExternalInput").ap()
    wpt_in = nc.dram_tensor("wpt", [128, 4, CODE], f16, kind="ExternalInput").ap()
    bp_in = nc.dram_tensor("bpc", [128, 2], f32, kind="ExternalInput").ap()
    qat_in = nc.dram_tensor("qat", [128, BPC, 2, MEM], f16, kind="ExternalInput").ap()
    qa4_in = nc.dram_tensor("qa4", [128, BPC, CODE], f16, kind="ExternalInput").ap()
    onesblk_in = nc.dram_tensor("onesblk", [128, 128], f32r, kind="ExternalInput").ap()
    zeros_in = nc.dram_tensor("zeros", [128, NT], f32r, kind="ExternalInput").ap()
    out_x = nc.dram_tensor("out_x", [T, 128, 2, NT], f16, kind="ExternalOutput").ap()
    out_u = nc.dram_tensor("out_u", [T, 128, 2, NT], f8, kind="ExternalOutput").ap()

    with tile.TileContext(nc) as tc, ExitStack() as ctx:
        consts = ctx.enter_context(tc.tile_pool(name="consts", bufs=1))
        fpool = ctx.enter_context(tc.tile_pool(name="fpool", bufs=4))
        xpool = ctx.enter_context(tc.tile_pool(name="xpool", bufs=6))
        rpool = ctx.enter_context(tc.tile_pool(name="rpool", bufs=2))
        npool = ctx.enter_context(tc.tile_pool(name="npool", bufs=2))
        upool = ctx.enter_context(tc.tile_pool(name="upool", bufs=4))
        ppp = ctx.enter_context(tc.tile_pool(name="ppp", bufs=2, space="PSUM"))
        plg = ctx.enter_context(tc.tile_pool(name="plg", bufs=3, space="PSUM"))
        pcs = ctx.enter_context(tc.tile_pool(name="pcs", bufs=1, space="PSUM"))
        ppu = ctx.enter_context(tc.tile_pool(name="ppu", bufs=2, space="PSUM"))

        wpt_sb = consts.tile([128, 4, CODE], f16, name="wpt_sb")
        bp_sb = consts.tile([128, 2], f32, name="bp_sb")
        qat_sb = consts.tile([128, BPC, 2, MEM], f16, name="qat_sb")
        qa4_sb = consts.tile([128, BPC, CODE], f16, name="qa4_sb")
        onesblk_sb = consts.tile([128, 128], f32r, name="onesblk_sb")
        # two persistent pexp buffers, zero-initialized once so the rows
        # outside the written 20-row strips stay 0 for the block matmul
        pexp_ab = [consts.tile([128, NT], f32r, name=f"pexp{i}") for i in range(2)]
        # constants ride the scalar ring so the sync ring starts on feats
        nc.scalar.dma_start(wpt_sb[:], wpt_in[:])
        nc.scalar.dma_start(bp_sb[:], bp_in[:])
        nc.scalar.dma_start(qat_sb[:], qat_in[:])
        nc.scalar.dma_start(qa4_sb[:], qa4_in[:])
        nc.scalar.dma_start(onesblk_sb[:], onesblk_in[:])
        nc.scalar.dma_start(pexp_ab[0][:], zeros_in[:])
        nc.scalar.dma_start(pexp_ab[1][:], zeros_in[:])

        x_t = {}
        lg_t = {}
        cs_t = {}
        pn_t = {}
        u_t = {}

        def bj(c):
            return c // NCH, c % NCH

        def glast(g):
            return min(GSZ * g + GSZ - 1, T - 1)

        def proj_chunk(c):
            ft = fpool.tile([128, 4, NT], f16, tag="ft", name=f"ft{c}")
            eng = nc.sync if c % 2 == 0 else nc.gpsimd
            eng.dma_start(ft[:], feats_in[c])
            xt = xpool.tile([128, 2, NT], f16, tag="xt", name=f"xt{c}")
            x_t[c] = xt
            for h in range(2):
                ps = ppp.tile([128, NT], f32, tag="proj_ps", name=f"pps{c}_{h}")
                for kk in range(4):
                    nc.tensor.matmul(
                        ps[:], wpt_sb[:, kk, h * 128:(h + 1) * 128], ft[:, kk, :],
                        start=(kk == 0), stop=(kk == 3))
                if h == 0:
                    nc.scalar.activation(
                        xt[:, 0, :], ps[:],
                        mybir.ActivationFunctionType.Identity,
                        bias=bp_sb[:, 0:1])
                else:
                    nc.vector.tensor_scalar_add(xt[:, 1, :], ps[:], bp_sb[:, 1:2])

        def x_flush(c):
            nc.gpsimd.dma_start(out_x[c], x_t[c][:])

        def logit_stage(c):
            b, _ = bj(c)
            lg = plg.tile([MEM, NT], f32, tag="lg", name=f"lg{c}")
            lg_t[c] = lg
            xt = x_t[c]
            for kk in range(2):
                nc.tensor.matmul(lg[:], qat_sb[:, b, kk, :], xt[:, kk, :],
                                 start=(kk == 0), stop=(kk == 1))

        def exp_stage(c):
            g, r = c // GSZ, c % GSZ
            pexp = pexp_ab[g % 2]
            nc.scalar.activation(pexp[32 * r:32 * r + MEM, :], lg_t.pop(c)[:],
                                 mybir.ActivationFunctionType.Exp)

        def colsum_group(g):
            cs = pcs.tile([128, NT], f32, tag="cs", name=f"cs{g}")
            nc.tensor.matmul(cs[:], onesblk_sb[:], pexp_ab[g % 2][:],
                             start=True, stop=True)
            cs_t[g] = cs

        def recip_pn_group(g):
            rc = rpool.tile([128, NT], f32, tag="rc", name=f"rc{g}")
            nc.vector.reciprocal_approx_fast(out=rc[:], in_=cs_t.pop(g)[:])
            pn = npool.tile([128, NT], f16, tag="pn", name=f"pn{g}")
            nc.gpsimd.tensor_mul(pn[:], pexp_ab[g % 2][:], rc[:])
            pn_t[g] = pn

        def u_stage(c):
            b, _ = bj(c)
            g, r = c // GSZ, c % GSZ
            pn = pn_t[g]
            s = slice(32 * r, 32 * r + MEM)
            ut = upool.tile([128, 2, NT], f8, tag="ut", name=f"ut{c}")
            u_t[c] = ut
            for h in range(2):
                us = ppu.tile([128, NT], f32, tag="u_ps", name=f"us{c}_{h}")
                nc.tensor.matmul(us[:], qa4_sb[s, b, h * 128:(h + 1) * 128],
                                 pn[s, :], start=True, stop=True)
                if h == 0:
                    nc.scalar.copy(ut[:, 0, :], us[:])
                else:
                    nc.vector.tensor_copy(ut[:, 1, :], us[:])

        def u_flush(c):
            nc.gpsimd.dma_start(out_u[c], u_t.pop(c)[:])

        # Chunk-level software pipeline; the group stages fire a couple
        # of iterations after the last member chunk's exp is emitted.
        colsum_at = {glast(g) + 4: g for g in range(NG)}
        recip_at = {glast(g) + 5: g for g in range(NG)}
        for t in range(T + 11):
            if t < T:
                proj_chunk(t)
            if 1 <= t and t - 1 < T:
                x_flush(t - 1)
            if 2 <= t and t - 2 < T:
                logit_stage(t - 2)
            if 3 <= t and t - 3 < T:
                exp_stage(t - 3)
            if t in colsum_at:
                colsum_group(colsum_at[t])
            if t in recip_at:
                recip_pn_group(recip_at[t])
            if 9 <= t and t - 9 < T:
                u_stage(t - 9)
            if 10 <= t and t - 10 < T:
                u_flush(t - 10)

    nc.compile()
    return nc


def kernel(feats, preds, labels, flag, queue, Wp, bp):
    from concourse.bass_utils import run_bass_kernel_spmd
    global LAST_RESULTS

    feats = np.ascontiguousarray(np.asarray(feats, dtype=np.float32))
    preds = np.ascontiguousarray(np.asarray(preds, dtype=np.float32))
    labels = np.asarray(labels).astype(np.int64)
    queue = np.ascontiguousarray(np.asarray(queue, dtype=np.float32))
    Wp = np.ascontiguousarray(np.asarray(Wp, dtype=np.float32))
    bp = np.ascontiguousarray(np.asarray(bp, dtype=np.float32))
    try:
        flag_v = int(np.asarray(flag))
    except TypeError:
        flag_v = int(flag)

    qfin = _host_queue_update(feats, preds, labels, flag_v, queue, Wp, bp)
    qA = qfin[labels]                                            # [B, 20, 256]
    # qat: [128, b, kk, m] with code c = kk*128 + p
    qat = np.ascontiguousarray(
        qA.transpose(0, 2, 1).reshape(B, 2, 128, MEM).transpose(2, 0, 1, 3)
        .astype(np.float16))                                     # [128, B, 2, 20]
    # qa4: [128, b, c] replicated at partition offsets 0/32/64(/96)
    qa4 = np.zeros((4, 32, B, CODE), dtype=np.float16)
    qa4[:, :MEM] = qA.transpose(1, 0, 2)[None].astype(np.float16)
    qa4 = qa4.reshape(128, B, CODE)
    wpt = np.ascontiguousarray(
        Wp.T.reshape(4, 128, CODE).transpose(1, 0, 2).astype(np.float16))
    bpc = np.ascontiguousarray(bp.reshape(2, 128).T)
    # block-diagonal ones: 1 where row strip == col strip (32-row blocks)
    blk = np.arange(128) // 32
    onesblk = (blk[:, None] == blk[None, :]).astype(np.float32)
    zeros = np.zeros((128, NT), dtype=np.float32)

    # feats: [b, c, hw] -> chunk-major [t=b*8+j, p, kk, n], c = kk*128+p
    f16 = (feats.reshape(B, 4, 128, NCH, NT).transpose(0, 3, 2, 1, 4)
           .astype(np.float16))                                  # [B, 8, 128, 4, NT]

    if "prog" not in _PROGRAM_CACHE:
        _PROGRAM_CACHE["prog"] = _build_program()
    nc = _PROGRAM_CACHE["prog"]

    in_maps = []
    for k in range(NCORES):
        s = slice(k * BPC, (k + 1) * BPC)
        in_maps.append({
            "feats": np.ascontiguousarray(f16[s]).reshape(T, 128, 4, NT),
            "wpt": wpt,
            "bpc": bpc,
            "qat": np.ascontiguousarray(qat[:, s]),
            "qa4": np.ascontiguousarray(qa4[:, s]),
            "onesblk": onesblk,
            "zeros": zeros,
        })

    trace = bool(int(os.environ.get("KERNEL_TRACE", "0")))
    tc_env = os.environ.get("KERNEL_TRACE_CORES", "")
    trace_cores = [int(x) for x in tc_env.split(",") if x] or None
    res = run_bass_kernel_spmd(nc, in_maps, core_ids=list(range(NCORES)),
                               trace=trace, trace_cores=trace_cores)
    LAST_RESULTS = res

    out = np.empty((B, 2 * CODE, HW), dtype=np.float32)
    for k in range(NCORES):
        xk = res.results[k]["out_x"]          # [T, 128, 2, NT] fp16
        uk = res.results[k]["out_u"]          # [T, 128, 2, NT] fp8
        xk = (xk.reshape(BPC, NCH, 128, 2, NT).transpose(0, 3, 2, 1, 4)
              .reshape(BPC, CODE, HW).astype(np.float32))
        uk = (uk.astype(np.float32).reshape(BPC, NCH, 128, 2, NT)
              .transpose(0, 3, 2, 1, 4).reshape(BPC, CODE, HW))
        s = slice(k * BPC, (k + 1) * BPC)
        out[s, CODE:] = xk
        out[s, :CODE] = uk
    return out.reshape(B, 2 * CODE, H, W_SP)


if __name__ == "__main__":
    d = np.load("/tmp/inputs.npz")
    out = kernel(d["feats"], d["preds"], d["labels"], d["flag"], d["queue"], d["Wp"], d["bp"])
    exp = np.load("/tmp/expected.npy")
    err = np.abs(out - exp)
    print("absmax err:", err.max(), "scale-rel:", err.max() / np.abs(exp).max())


# revision 17
# speedup vs baseline: 1.4844x; 1.1887x over previous
"""TRN2 Bass kernel for nn_ClassSemantic (scatter_memory).

Strategy (v2)
-------------
Data-parallel over batch: core k owns samples 4k..4k+3 and runs
projection (1x1 conv) + memory-gather attention + concat on device.

The kernel is memory-regime: the fp32 baseline was at ~95% of the DMA
roofline (67 MB/core).  v2 cuts HBM bytes ~2.3x:
  - feats are sent as bf16 (host converts; fp16 is NOT used because the
    PE streams fp16 moving operands at 2 cycles/col vs bf16's 1)
  - the x output half is written as bf16 (~4e-3 rel err vs 2e-2 gate)
  - the attention output half u is written as fp8-e4m3 (|u| <= 0.08
    while the output scale is 5.7, so fp8 error is ~1e-4 of scale)
Host packs feats into the exact per-chunk SBUF layout so each chunk is
ONE contiguous 512KB DMA descriptor (descriptor issue costs ~0.7us of
engine time each; baseline used 5/chunk, v2 uses 3/chunk).

With bytes halved the PE becomes the critical engine (~84us); the
attention path is arranged to minimize PE column passes:
  - logits per chunk land in [20, NT] PSUM at base partition 0; the
    exp ACTIVATE partition-shifts its write into a persistent
    [128, NT] tile at offset 32*(c%3), batching 3 chunks per group
    (engines are column-bound, so ops on [20, NT] cost the same as
    [128, NT]; matmuls may NOT write PSUM at partition offsets, but
    engine writes and matmul READS at offsets 0/32/64 are legal)
  - the softmax column-sum for a whole group is ONE [128,128]
    block-diagonal ones matmul (rows outside the 20 valid slots of
    each 32-row strip are zeroed by a one-time zeros DMA)
  - recip + attn-normalize run once per group on [128, NT]
  - the u matmuls contract the 20-row strip at offset 32*(c%3)

The sequential EMA queue update depends only on per-sample masked
feature means, computed on host (0.4% of FLOPs), and the final queue
rows are shipped to every core as constants; no collective needed.
"""
import os
import numpy as np
import ml_dtypes
from contextlib import ExitStack

B, IN_C, H, W_SP = 32, 512, 64, 64
CODE, CLASSES, MEM = 256, 4, 20
HW = H * W_SP              # 4096
NCORES = 8
BPC = B // NCORES          # 4 samples per core
DECAY, EPS = 0.9, 1e-12
NCH = 8                    # chunks per sample
NT = HW // NCH             # 512 spatial positions per chunk
T = BPC * NCH              # 32 chunks per core
GSZ = 3                    # chunks per softmax group (offsets 0/32/64)
NG = (T + GSZ - 1) // GSZ
F8 = ml_dtypes.float8_e4m3

_PROGRAM_CACHE = {}
LAST_RESULTS = None        # stash for test harness introspection


def _host_queue_update(feats, preds, labels, flag, queue, Wp, bp):
    """Final queue after the reference's sequential EMA scan (float64)."""
    if int(flag) != 1:
        return queue.astype(np.float32)
    f3 = feats.reshape(B, IN_C, HW)
    p2 = preds.reshape(B, HW)
    g = np.matmul(f3, p2[:, :, None])[:, :, 0] / np.float32(HW)
    feat = g @ Wp.T.astype(np.float32) + bp[None, :] * p2.mean(axis=1)[:, None]
    q = queue.astype(np.float64)
    for i in range(B):
        l = int(labels[i])
        f = feat[i].astype(np.float64)
        slot = q[l]
        logit = slot @ f
        upd = logit[:, None] * f[None, :]
        nrm = np.sqrt((upd * upd).sum(axis=1, keepdims=True))
        upd = upd / np.maximum(nrm, EPS)
        q[l] = DECAY * slot + (1.0 - DECAY) * upd
    return q.astype(np.float32)


def _build_program():
    from concourse import bacc, mybir
    import concourse.tile as tile

    f32, f32r = mybir.dt.float32, mybir.dt.float32r
    f16, f8 = mybir.dt.bfloat16, mybir.dt.float8e4
    nc = bacc.Bacc("TRN2", target_bir_lowering=False, debug=False)

    feats_in = nc.dram_tensor("feats", [T, 128, 4, NT], f16, kind="ExternalInput").ap()
    wpt_in = nc.dram_tensor("wpt", [128, 4, CODE], f16, kind="ExternalInput").ap()
    bp_in = nc.dram_tensor("bpc", [128, 2], f32, kind="ExternalInput").ap()
    qat_in = nc.dram_tensor("qat", [128, BPC, 2, MEM], f16, kind="ExternalInput").ap()
    qa4_in = nc.dram_tensor("qa4", [128, BPC, CODE], f16, kind="ExternalInput").ap()
    onesblk_in = nc.dram_tensor("onesblk", [128, 128], f32r, kind="ExternalInput").ap()
    zeros_in = nc.dram_tensor("zeros", [128, NT], f32r, kind="ExternalInput").ap()
    out_x = nc.dram_tensor("out_x", [T, 128, 2, NT], f16, kind="ExternalOutput").ap()
    out_u = nc.dram_tensor("out_u", [T, 128, 2, NT], f8, kind="ExternalOutput").ap()

    with tile.TileContext(nc) as tc, ExitStack() as ctx:
        consts = ctx.enter_context(tc.tile_pool(name="consts", bufs=1))
        fpool = ctx.enter_context(tc.tile_pool(name="fpool", bufs=4))
        xpool = ctx.enter_context(tc.tile_pool(name="xpool", bufs=6))
        rpool = ctx.enter_context(tc.tile_pool(name="rpool", bufs=2))
        npool = ctx.enter_context(tc.tile_pool(name="npool", bufs=2))
        upool = ctx.enter_context(tc.tile_pool(name="upool", bufs=4))
        ppp = ctx.enter_context(tc.tile_pool(name="ppp", bufs=2, space="PSUM"))
        plg = ctx.enter_context(tc.tile_pool(name="plg", bufs=3, space="PSUM"))
        pcs = ctx.enter_context(tc.tile_pool(name="pcs", bufs=1, space="PSUM"))
        ppu = ctx.enter_context(tc.tile_pool(name="ppu", bufs=2, space="PSUM"))

        wpt_sb = consts.tile([128, 4, CODE], f16, name="wpt_sb")
        bp_sb = consts.tile([128, 2], f32, name="bp_sb")
        qat_sb = consts.tile([128, BPC, 2, MEM], f16, name="qat_sb")
        qa4_sb = consts.tile([128, BPC, CODE], f16, name="qa4_sb")
        onesblk_sb = consts.tile([128, 128], f32r, name="onesblk_sb")
        # two persistent pexp buffers, zero-initialized once so the rows
        # outside the written 20-row strips stay 0 for the block matmul
        pexp_ab = [consts.tile([128, NT], f32r, name=f"pexp{i}") for i in range(2)]
        # constants ride the scalar ring so the sync ring starts on feats
        nc.scalar.dma_start(wpt_sb[:], wpt_in[:])
        nc.scalar.dma_start(bp_sb[:], bp_in[:])
        nc.scalar.dma_start(qat_sb[:], qat_in[:])
        nc.scalar.dma_start(qa4_sb[:], qa4_in[:])
        nc.scalar.dma_start(onesblk_sb[:], onesblk_in[:])
        nc.scalar.dma_start(pexp_ab[0][:], zeros_in[:])
        nc.scalar.dma_start(pexp_ab[1][:], zeros_in[:])

        x_t = {}
        lg_t = {}
        cs_t = {}
        pn_t = {}
        u_t = {}

        def bj(c):
            return c // NCH, c % NCH

        def glast(g):
            return min(GSZ * g + GSZ - 1, T - 1)

        def proj_chunk(c):
            ft = fpool.tile([128, 4, NT], f16, tag="ft", name=f"ft{c}")
            eng = nc.sync if c % 2 == 0 else nc.scalar
            eng.dma_start(ft[:], feats_in[c])
            xt = xpool.tile([128, 2, NT], f16, tag="xt", name=f"xt{c}")
            x_t[c] = xt
            for h in range(2):
                ps = ppp.tile([128, NT], f32, tag="proj_ps", name=f"pps{c}_{h}")
                for kk in range(4):
                    nc.tensor.matmul(
                        ps[:], wpt_sb[:, kk, h * 128:(h + 1) * 128], ft[:, kk, :],
                        start=(kk == 0), stop=(kk == 3))
                if h == 0:
                    nc.scalar.activation(
                        xt[:, 0, :], ps[:],
                        mybir.ActivationFunctionType.Identity,
                        bias=bp_sb[:, 0:1])
                else:
                    nc.vector.tensor_scalar_add(xt[:, 1, :], ps[:], bp_sb[:, 1:2])

        def x_flush(c):
            nc.gpsimd.dma_start(out_x[c], x_t[c][:])

        def logit_stage(c):
            b, _ = bj(c)
            lg = plg.tile([MEM, NT], f32, tag="lg", name=f"lg{c}")
            lg_t[c] = lg
            xt = x_t[c]
            for kk in range(2):
                nc.tensor.matmul(lg[:], qat_sb[:, b, kk, :], xt[:, kk, :],
                                 start=(kk == 0), stop=(kk == 1))

        def exp_stage(c):
            g, r = c // GSZ, c % GSZ
            pexp = pexp_ab[g % 2]
            nc.scalar.activation(pexp[32 * r:32 * r + MEM, :], lg_t.pop(c)[:],
                                 mybir.ActivationFunctionType.Exp)

        def colsum_group(g):
            cs = pcs.tile([128, NT], f32, tag="cs", name=f"cs{g}")
            nc.tensor.matmul(cs[:], onesblk_sb[:], pexp_ab[g % 2][:],
                             start=True, stop=True)
            cs_t[g] = cs

        def recip_pn_group(g):
            rc = rpool.tile([128, NT], f32, tag="rc", name=f"rc{g}")
            nc.vector.reciprocal_approx_fast(out=rc[:], in_=cs_t.pop(g)[:])
            pn = npool.tile([128, NT], f16, tag="pn", name=f"pn{g}")
            nc.gpsimd.tensor_mul(pn[:], pexp_ab[g % 2][:], rc[:])
            pn_t[g] = pn

        def u_stage(c):
            b, _ = bj(c)
            g, r = c // GSZ, c % GSZ
            pn = pn_t[g]
            s = slice(32 * r, 32 * r + MEM)
            ut = upool.tile([128, 2, NT], f8, tag="ut", name=f"ut{c}")
            u_t[c] = ut
            for h in range(2):
                us = ppu.tile([128, NT], f32, tag="u_ps", name=f"us{c}_{h}")
                nc.tensor.matmul(us[:], qa4_sb[s, b, h * 128:(h + 1) * 128],
                                 pn[s, :], start=True, stop=True)
                if h == 0:
                    nc.scalar.copy(ut[:, 0, :], us[:])
                else:
                    nc.vector.tensor_copy(ut[:, 1, :], us[:])

        def u_flush(c):
            nc.gpsimd.dma_start(out_u[c], u_t.pop(c)[:])

        # Chunk-level software pipeline; the group stages fire a couple
        # of iterations after the last member chunk's exp is emitted.
        colsum_at = {glast(g) + 4: g for g in range(NG)}
        recip_at = {glast(g) + 5: g for g in range(NG)}
        for t in range(T + 10):
            if t < T:
                proj_chunk(t)
            if 1 <= t and t - 1 < T:
                x_flush(t - 1)
            if 2 <= t and t - 2 < T:
                logit_stage(t - 2)
            if 3 <= t and t - 3 < T:
                exp_stage(t - 3)
            if t in colsum_at:
                colsum_group(colsum_at[t])
            if t in recip_at:
                recip_pn_group(recip_at[t])
            if 8 <= t and t - 8 < T:
                u_stage(t - 8)
            if 9 <= t and t - 9 < T:
                u_flush(t - 9)

    nc.compile()
    return nc


def kernel(feats, preds, labels, flag, queue, Wp, bp):
    from concourse.bass_utils import run_bass_kernel_spmd
    global LAST_RESULTS

    feats = np.ascontiguousarray(np.asarray(feats, dtype=np.float32))
    preds = np.ascontiguousarray(np.asarray(preds, dtype=np.float32))
    labels = np.asarray(labels).astype(np.int64)
    queue = np.ascontiguousarray(np.asarray(queue, dtype=np.float32))
    Wp = np.ascontiguousarray(np.asarray(Wp, dtype=np.float32))
    bp = np.ascontiguousarray(np.asarray(bp, dtype=np.float32))
    try:
        flag_v = int(np.asarray(flag))
    except TypeError:
        flag_v = int(flag)

    qfin = _host_queue_update(feats, preds, labels, flag_v, queue, Wp, bp)
    qA = qfin[labels]                                            # [B, 20, 256]
    # qat: [128, b, kk, m] with code c = kk*128 + p
    qat = np.ascontiguousarray(
        qA.transpose(0, 2, 1).reshape(B, 2, 128, MEM).transpose(2, 0, 1, 3)
        .astype(ml_dtypes.bfloat16))                             # [128, B, 2, 20]
    # qa4: [128, b, c] replicated at partition offsets 0/32/64(/96)
    qa4 = np.zeros((4, 32, B, CODE), dtype=ml_dtypes.bfloat16)
    qa4[:, :MEM] = qA.transpose(1, 0, 2)[None].astype(ml_dtypes.bfloat16)
    qa4 = qa4.reshape(128, B, CODE)
    wpt = np.ascontiguousarray(
        Wp.T.reshape(4, 128, CODE).transpose(1, 0, 2).astype(ml_dtypes.bfloat16))
    bpc = np.ascontiguousarray(bp.reshape(2, 128).T)
    # block-diagonal ones: 1 where row strip == col strip (32-row blocks)
    blk = np.arange(128) // 32
    onesblk = (blk[:, None] == blk[None, :]).astype(np.float32)
    zeros = np.zeros((128, NT), dtype=np.float32)

    # feats: [b, c, hw] -> chunk-major [t=b*8+j, p, kk, n], c = kk*128+p
    f16 = (feats.reshape(B, 4, 128, NCH, NT).transpose(0, 3, 2, 1, 4)
           .astype(ml_dtypes.bfloat16))                          # [B, 8, 128, 4, NT]

    if "prog" not in _PROGRAM_CACHE:
        _PROGRAM_CACHE["prog"] = _build_program()
    nc = _PROGRAM_CACHE["prog"]

    in_maps = []
    for k in range(NCORES):
        s = slice(k * BPC, (k + 1) * BPC)
        in_maps.append({
            "feats": np.ascontiguousarray(f16[s]).reshape(T, 128, 4, NT),
            "wpt": wpt,
            "bpc": bpc,
            "qat": np.ascontiguousarray(qat[:, s]),
            "qa4": np.ascontiguousarray(qa4[:, s]),
            "onesblk": onesblk,
            "zeros": zeros,
        })

    trace = bool(int(os.environ.get("KERNEL_TRACE", "0")))
    tc_env = os.environ.get("KERNEL_TRACE_CORES", "")
    trace_cores = [int(x) for x in tc_env.split(",") if x] or None
    res = run_bass_kernel_spmd(nc, in_maps, core_ids=list(range(NCORES)),
                               trace=trace, trace_cores=trace_cores)
    LAST_RESULTS = res

    out = np.empty((B, 2 * CODE, HW), dtype=np.float32)
    for k in range(NCORES):
        xk = res.results[k]["out_x"]          # [T, 128, 2, NT] bf16
        uk = res.results[k]["out_u"]          # [T, 128, 2, NT] fp8
        xk = (xk.reshape(BPC, NCH, 128, 2, NT).transpose(0, 3, 2, 1, 4)
              .reshape(BPC, CODE, HW).astype(np.float32))
        uk = (uk.astype(np.float32).reshape(BPC, NCH, 128, 2, NT)
              .transpose(0, 3, 2, 1, 4).reshape(BPC, CODE, HW))
        s = slice(k * BPC, (k + 1) * BPC)
        out[s, CODE:] = xk
        out[s, :CODE] = uk
    return out.reshape(B, 2 * CODE, H, W_SP)


if __name__ == "__main__":
    d = np.load("/tmp/inputs.npz")
    out = kernel(d["feats"], d["preds"], d["labels"], d["flag"], d["queue"], d["Wp"], d["bp"])
    exp = np.load("/tmp/expected.npy")
    err = np.abs(out - exp)
    print("absmax err:", err.max(), "scale-rel:", err.max() / np.abs(exp).max())
